# revision 5
# baseline (speedup 1.0000x reference)
"""Kent-distribution pairwise KLD loss kernel for Trainium2 (8 NeuronCores).

v3: bf16 single-pass matmul with exact hi/lo (split-float) features.

The [N, M] pairwise KLD matrix factors exactly as a rank-11 product
U @ V^T.  The fp32 matmul (4 cyc/col x 2 PE passes) is reformulated as
ONE bf16 matmul (1 cyc/col) with contraction K=44: each fp32 feature x
is bf16 hi + bf16 lo (lo = bf16(x - hi), hi+lo == x to ~2^-17 rel) and

  U44 = [Uh; Ul; Uh; Ul],  V44 = [Vh; Vh; Vl; Vl]
  sum_k U44[k] V44[k] = sum_f (Uh+Ul)(Vh+Vl) = U . V  (all cross terms)

VT is stored m-major (strided copy out of the transpose PSUM) so the
matmul moving operand is contiguous and each 512-col chunk depends on
exactly one PSUM->SBUF copy.

Numerics (vs jax reference):
 - l1 = (k^2-k-4b^2)/D, l2 = ((k-1)k^2 - ks - s/2)/D^2, D = k^2-4b^2,
   s = 4b^2 (exact ratios; exp(-EPS) ~ 1-1e-6 dropped).
 - |gamma1|^2 == 1 exactly => kappa_a.Ex_a = k*l1.
 - l2 * sum(dVdiag) == 0 (|g_b2|=|g_b3|=1) => l2 dropped from UF[5:8].
 - LN_2PI cancels between c_b and -c_a => dropped from both.
 - G3 := -gamma3 throughout (only quadratic uses; sign-insensitive).
 - Sin HW domain is [-pi,pi]: cos(x) = sin(pi/2 - |x|).

Elementwise chain is packed via strided APs over one workspace tile W
(slot axis x 18 group columns); target groups live in columns 2:18,
pred groups in columns 0:2 (slots can overlap when columns differ).
Three dummy activations (sin / sin+bias / ln) run first so all ACT
table loads hide under the input-DMA latency.
"""

import sys

import numpy as np

sys.path.insert(0, "/opt/trn_rl_repo")

import concourse.bass as bass  # noqa: E402,F401
import concourse.mybir as mybir  # noqa: E402
import concourse.tile as tile  # noqa: E402
from concourse import bacc  # noqa: E402
from concourse.masks import make_identity  # noqa: E402

F32 = mybir.dt.float32
BF16 = mybir.dt.bfloat16
AF = mybir.ActivationFunctionType
ALU = mybir.AluOpType

N = 2048
M = 2048
NCORES = 8
NS = N // NCORES  # 256 pred rows per core
K = 11  # fp32 feature rank
K4 = 4 * K  # bf16 hi/lo doubled contraction
GP = NS // 128  # pred row-groups (2)
GT = M // 128  # target row-groups (16)
G = GP + GT  # 18

PI = float(np.pi)
EPS = 1e-6


def _body(tc, pred, targ, out):
    nc = tc.nc
    with (
        tc.tile_pool(name="main", bufs=1) as pool,
        tc.tile_pool(name="vt_psum", bufs=4, space="PSUM") as vpp,
        tc.tile_pool(name="ut_psum", bufs=1, space="PSUM") as upp,
        tc.tile_pool(name="out_psum", bufs=3, space="PSUM") as opp,
    ):
        def t(shape, tag, dtype=F32):
            return pool.tile([128, *shape], dtype, name=tag, tag=tag)

        dve = nc.vector
        act = nc.scalar
        gps = nc.gpsimd

        # ---- input DMAs first: pred partition p holds rows 2p,2p+1; targ
        # partition p holds rows 16p..16p+15 (per-partition contiguous).
        params = t([G * 5], "params")
        nc.sync.dma_start(
            out=params[:, 0 : GP * 5],
            in_=pred.rearrange("(p j) c -> p (j c)", p=128),
        )
        act.dma_start(
            out=params[:, GP * 5 : G * 5],
            in_=targ.rearrange("(p j) c -> p (j c)", p=128),
        )

        P5 = params.rearrange("p (g c) -> p c g", c=5)  # [128, 5, 18]
        kap = P5[:, 3, :]
        bet = P5[:, 4, :]
        kap_p, bet_p = kap[:, 0:GP], bet[:, 0:GP]
        kap_t, bet_t = kap[:, GP:G], bet[:, GP:G]

        # ---- constants (overlap the DMA transfer)
        half_pi = pool.tile([128, 1], F32, name="half_pi", tag="half_pi")
        gps.memset(half_pi, PI / 2)
        eps_c = pool.tile([128, 1], F32, name="eps_c", tag="eps_c")
        gps.memset(eps_c, EPS)
        one_c = pool.tile([128, 1], F32, name="one_c", tag="one_c")
        gps.memset(one_c, 1.0)
        ident = pool.tile([128, 128], BF16, name="ident", tag="ident")
        make_identity(nc, ident)

        # dummy activations: force every ACT table (sin / sin-variant / ln)
        # to load while the input DMA is in flight
        dmy = pool.tile([128, 3], F32, name="dmy", tag="dmy")
        act.activation(dmy[:, 0:1], half_pi[:], AF.Sin)
        act.activation(dmy[:, 1:2], half_pi[:], AF.Sin, bias=half_pi, scale=-1.0)
        act.activation(dmy[:, 2:3], one_c[:], AF.Ln, bias=eps_c)

        # ---- workspace W: slot axis x 18 group columns.
        # 0 ce, 1 ca, 2 cp | 3 se, 4 sa, 5 sp | 6 g1x, 7 g1y, 8 g1z |
        # 9 m2, 10 m4 | 11 spce, 12 spse, 13 cpce, 14 cpse |
        # 15 m2ce, 16 m2se, 17 m4ce, 18 m4se |
        # 19 g2x, 20 g2y, 21 g2z | 22 G3x, 23 G3y, 24 G3z |
        # 25:31 squares [g2,G3] (V cols) / 25:28 p1diag (U cols) |
        # 31:37 offdiags (V) / 28:31 p1off (U cols) |
        # 37:40 dVdiag, 40:43 dVoff | 43 km, 44 kp, 45 LNIN, 46 lnprod |
        # 47:58 VF (V cols; 47 = const 1) | 58:61 absv | 61 b2
        S = 62
        W = t([S, G], "W")
        gps.memset(W[:, 47, :], 1.0)

        # ---- Pool: kappa/beta + rec-independent U-chain (needs only DMA)
        U = t([20, GP], "U")
        b2p = W[:, 61, 0:GP]
        one_b = one_c.broadcast_to([128, GP])

        def u(i):
            return U[:, i, :]

        gps.tensor_add(W[:, 61, :], bet, bet)  # b2 = 2*beta
        gps.tensor_sub(W[:, 43, :], kap, W[:, 61, :])  # km
        gps.tensor_add(W[:, 44, :], kap, W[:, 61, :])  # kp
        gps.tensor_mul(W[:, 45, :], W[:, 43, :], W[:, 44, :])  # LNIN
        gps.tensor_mul(u(0), kap_p, kap_p)  # x2 = k^2
        gps.tensor_mul(u(1), b2p, b2p)  # s = 4b^2
        gps.tensor_sub(u(2), u(0), u(1))  # D = k^2 - s
        gps.tensor_sub(u(5), u(2), kap_p)  # n2 = k^2-k-s = D - k
        gps.tensor_sub(u(7), kap_p, one_b)  # k-1
        gps.tensor_mul(u(8), u(7), u(0))  # t2 = (k-1)k^2
        gps.tensor_mul(u(9), kap_p, u(1))  # ks
        gps.tensor_mul(u(10), bet_p, b2p)  # s/2 = 2b^2
        gps.tensor_sub(u(11), u(8), u(9))
        gps.tensor_sub(u(12), u(11), u(10))  # Qh = t2 - ks - s/2

        # ---- trig (ACT) + DVE gamma chain ----
        angles = P5[:, 0:3, :]
        absv = W[:, 58:61, :]
        dve.scalar_tensor_tensor(absv, angles, -1.0, angles, ALU.mult, ALU.max)
        act.activation(W[:, 3:6, :], angles, AF.Sin)  # se, sa, sp
        act.activation(W[:, 0:3, :], absv, AF.Sin, bias=half_pi, scale=-1.0)
        # lnprod = ln((k-2b)(k+2b) + EPS)
        act.activation(W[:, 46, :], W[:, 45, :], AF.Ln, bias=eps_c)

        def rep_outer(ap, n):  # [128, a, g] -> [128, a, n, g]
            return ap.unsqueeze(2).broadcast_to([128, ap.shape[1], n, ap.shape[2]])

        def rep_inner(ap, n):  # [128, b, g] -> [128, n, b, g]
            return ap.unsqueeze(1).broadcast_to([128, n, ap.shape[1], ap.shape[2]])

        cpsp = W[:, 2:6:3, :]  # [cp, sp]
        cese = W[:, 0:4:3, :]  # [ce, se]
        sa_b = W[:, 4:5, :].broadcast_to([128, 2, G])
        dve.tensor_mul(W[:, 9:11, :], cpsp, W[:, 1:2, :].broadcast_to([128, 2, G]))
        dve.tensor_copy(W[:, 6:7, :], W[:, 1:2, :])  # g1x = ca
        dve.tensor_mul(W[:, 7:9, :], sa_b, cese)  # g1y, g1z
        aux1_out = W[:, 11:15, :].rearrange("p (a b) g -> p a b g", a=2)
        dve.tensor_mul(aux1_out, rep_outer(W[:, 5:1:-3, :], 2), rep_inner(cese, 2))
        aux2_out = W[:, 15:19, :].rearrange("p (a b) g -> p a b g", a=2)
        dve.tensor_mul(aux2_out, rep_outer(W[:, 9:11, :], 2), rep_inner(cese, 2))
        dve.scalar_tensor_tensor(W[:, 19:23:3, :], cpsp, -1.0, sa_b, ALU.mult, ALU.mult)
        dve.tensor_sub(W[:, 20:25:4, :], W[:, 15:19:3, :], W[:, 12:14, :])
        dve.tensor_add(W[:, 21:24:2, :], W[:, 16:18, :], W[:, 11:15:3, :])
        # U reciprocal slotted here: D has long been ready on Pool
        dve.reciprocal(u(3), u(2))

        # ---- V pair products (target columns) ----
        g6t = W[:, 19:25, GP:G]
        dve.tensor_mul(W[:, 25:31, GP:G], g6t, g6t)
        offa_in1 = W[:, 19:25, GP:G].rearrange("p (a b) g -> p a b g", a=2)[:, :, 1:3, :]
        offa_out = W[:, 31:37, GP:G].rearrange("p (a b) g -> p a b g", a=2)[:, :, 0:2, :]
        dve.tensor_mul(offa_out, rep_outer(W[:, 19:25:3, GP:G], 2), offa_in1)
        dve.tensor_mul(W[:, 33:39:3, GP:G], W[:, 20:26:3, GP:G], W[:, 21:27:3, GP:G])
        dve.tensor_sub(W[:, 37:40, GP:G], W[:, 28:31, GP:G], W[:, 25:28, GP:G])
        dve.tensor_sub(W[:, 40:43, GP:G], W[:, 34:37, GP:G], W[:, 31:34, GP:G])

        # ---- Pool: rec-dependent U-chain + p1 products ----
        gps.tensor_mul(u(6), u(5), u(3))  # l1
        gps.tensor_mul(u(13), u(3), u(3))  # rec^2
        gps.tensor_mul(u(14), u(12), u(13))  # l2
        gps.tensor_sub(u(15), u(6), u(14))  # dE
        gps.tensor_mul(u(16), kap_p, u(6))  # kadot = k*l1
        g1p = W[:, 6:9, 0:GP]
        gps.tensor_mul(W[:, 25:28, 0:GP], g1p, g1p)
        gps.tensor_mul(
            W[:, 28:30, 0:GP],
            W[:, 6:7, 0:GP].broadcast_to([128, 2, GP]),
            W[:, 7:9, 0:GP],
        )
        gps.tensor_mul(W[:, 30, 0:GP], W[:, 7, 0:GP], W[:, 8, 0:GP])

        # ---- V features (fp32) into W[47:58] ----
        kt3 = kap_t.unsqueeze(1).broadcast_to([128, 3, GT])
        bt3 = bet_t.unsqueeze(1).broadcast_to([128, 3, GT])
        dve.scalar_tensor_tensor(
            W[:, 48, GP:G], W[:, 46, GP:G], -0.5, kap_t, ALU.mult, ALU.add
        )
        dve.scalar_tensor_tensor(
            W[:, 49:52, GP:G], W[:, 6:9, GP:G], -1.0, kt3, ALU.mult, ALU.mult
        )
        dve.tensor_mul(W[:, 52:55, GP:G], W[:, 37:40, GP:G], bt3)
        dve.scalar_tensor_tensor(
            W[:, 55:58, GP:G], W[:, 40:43, GP:G], 2.0, bt3, ALU.mult, ALU.mult
        )

        # ---- U features (fp32) ----
        UF = t([K, GP], "UF")
        gps.memset(UF[:, 1, :], 1.0)
        l1b = U[:, 6:7, :].broadcast_to([128, 3, GP])
        deb = U[:, 15:16, :].broadcast_to([128, 3, GP])
        de2 = U[:, 15:16, :].broadcast_to([128, 2, GP])
        gps.tensor_mul(UF[:, 2:5, :], g1p, l1b)
        gps.tensor_mul(UF[:, 5:8, :], W[:, 25:28, 0:GP], deb)
        gps.tensor_mul(UF[:, 8:10, :], W[:, 28:30, 0:GP], de2)
        gps.tensor_mul(UF[:, 10, :], W[:, 30, 0:GP], U[:, 15, :])
        # A = 0.5*lnprod - k + k*l1 (a1 on DVE; LN_2PI dropped)
        dve.scalar_tensor_tensor(
            u(18), W[:, 46, 0:GP], 0.5, kap_p, ALU.mult, ALU.subtract
        )
        gps.tensor_add(UF[:, 0, :], u(18), u(16))

        # ---- hi/lo split to bf16 ----
        VH = t([K4, GT], "VH", BF16)
        UH = t([K4, GP], "UH", BF16)
        # U44 = [Uh; Ul; Uh; Ul]
        uh2 = UH.rearrange("p (r k) g -> p r k g", r=2)
        act.copy(uh2[:, :, 0:K, :], rep_inner(UF[:], 2))
        gps.tensor_sub(
            uh2[:, :, K : 2 * K, :], rep_inner(UF[:], 2), rep_inner(UH[:, 0:K, :], 2)
        )
        # V44 = [Vh; Vh; Vl; Vl], split per 4-group chunk to unblock the
        # transposes as early as possible
        for q in range(4):
            gs = GP + 4 * q
            VFq = W[:, 47:58, gs : gs + 4]
            vh_hi = VH[:, 0 : 2 * K, 4 * q : 4 * q + 4].rearrange(
                "p (r k) g -> p r k g", r=2
            )
            vh_lo = VH[:, 2 * K : 4 * K, 4 * q : 4 * q + 4].rearrange(
                "p (r k) g -> p r k g", r=2
            )
            act.copy(vh_hi, rep_inner(VFq, 2))
            dve.tensor_sub(
                vh_lo, rep_inner(VFq, 2), rep_inner(VH[:, 0:K, 4 * q : 4 * q + 4], 2)
            )

        # ---- PE: transposes + matmuls, pipelined per 4-group chunk ----
        # VT is m-major: col m = 16p + j; transpose q output col (jj, p)
        # scatters to m = 16p + 4q + jj via the strided copy below.
        VT = pool.tile([K4, M], BF16, name="VT", tag="VT")
        VTm = VT.rearrange("k (p j) -> k j p", j=GT)
        UT = pool.tile([K4, NS], BF16, name="UT", tag="UT")
        utp = upp.tile([K4, 1024], BF16, name="utp", tag="utp")
        outv = out.rearrange("(t p) m -> p t m", p=128)

        vtps = []
        for q in range(4):
            vtp = vpp.tile([K4, 1024], BF16, name="vtp", tag="vtp")
            vtps.append(vtp)
            for jj in range(4):
                j = q * 4 + jj
                nc.tensor.transpose(
                    vtp[:, jj * 128 : (jj + 1) * 128], VH[:, :, j], ident[:]
                )
            if q == 1:
                # U transposes + interleave copy (UT col = pred row 2p+j)
                for j in range(GP):
                    nc.tensor.transpose(
                        utp[:, j * 128 : (j + 1) * 128], UH[:, :, j], ident[:]
                    )
                act.copy(
                    UT.rearrange("k (p j) -> k j p", j=GP),
                    utp[:, 0 : GP * 128].rearrange("k (j p) -> k j p", p=128),
                )
            src = vtp[:, 0:512].rearrange("k (j p) -> k j p", p=128)
            if q % 2 == 0:
                dve.tensor_copy(VTm[:, 4 * q : 4 * q + 4, :], src)
            else:
                act.copy(VTm[:, 4 * q : 4 * q + 4, :], src)

        ci = 0
        for c in range(4):
            for ti in range(GP):
                ops = opp.tile([128, 512], F32, name="ops", tag="ops")
                nc.tensor.matmul(
                    ops[:],
                    UT[:, 128 * ti : 128 * (ti + 1)],
                    VT[:, 512 * c : 512 * (c + 1)],
                    start=True,
                    stop=True,
                )
                out_sb = pool.tile(
                    [128, 512], F32, name="out_sb", tag="out_sb", bufs=4
                )
                if ci % 2 == 0:
                    dve.tensor_copy(out_sb[:], ops[:])
                else:
                    act.copy(out_sb[:], ops[:])
                ring = nc.sync if ci % 2 == 0 else act
                ring.dma_start(
                    out=outv[:, ti, 512 * c : 512 * (c + 1)], in_=out_sb[:]
                )
                ci += 1


def build():
    nc = bacc.Bacc()
    pred = nc.dram_tensor("pred", [NS, 5], F32, kind="ExternalInput")
    targ = nc.dram_tensor("targ", [M, 5], F32, kind="ExternalInput")
    out = nc.dram_tensor("out", [NS, M], F32, kind="ExternalOutput")
    with tile.TileContext(nc) as tc:
        _body(tc, pred[:], targ[:], out[:])
    nc.finalize()
    return nc


_NC_CACHE = None


def _get_nc():
    global _NC_CACHE
    if _NC_CACHE is None:
        _NC_CACHE = build()
    return _NC_CACHE


def kernel(kent_pred, kent_target, trace=False, tmpdir=None):
    from concourse.bass_utils import run_bass_kernel_spmd

    nc = _get_nc()
    kent_pred = np.ascontiguousarray(np.asarray(kent_pred, dtype=np.float32))
    kent_target = np.ascontiguousarray(np.asarray(kent_target, dtype=np.float32))
    in_maps = [
        {"pred": kent_pred[i * NS : (i + 1) * NS], "targ": kent_target}
        for i in range(NCORES)
    ]
    res = run_bass_kernel_spmd(
        nc, in_maps, core_ids=list(range(NCORES)), trace=trace, tmpdir=tmpdir
    )
    out = np.concatenate([r["out"] for r in res.results], axis=0)
    if trace:
        kernel.last_results = res
    return out


# revision 6
# speedup vs baseline: 1.0137x; 1.0137x over previous
"""Kent-distribution pairwise KLD loss kernel for Trainium2 (8 NeuronCores).

v4: bf16 single-pass matmul with exact hi/lo (split-float) features.

KLD[N, M] factors exactly as rank-11 U @ V^T.  The fp32 matmul (4
cyc/col x 2 PE passes) becomes ONE bf16 matmul (1 cyc/col) with K=44:
each fp32 feature x is bf16 hi + bf16 lo (hi+lo == x to ~2^-17) and

  U44 = [Uh; Ul; Uh; Ul],  V44 = [Vh; Vh; Vl; Vl]
  sum_k U44[k] V44[k] = sum_f (Uh+Ul)(Vh+Vl) = U . V

Scheduling notes (from trace analysis):
 - Only one ACT table set is resident at a time; every Sin<->Ln switch
   reloads (1.28us).  ACT runs [dummy sin] sin cos [ln load] ln, then
   only table-free COPYs.
 - Engine queues are in-order: the U-side reciprocal is issued on DVE
   after the gamma chain (its Pool-side input is long ready), not
   before (v2 stalled DVE 2us on it).
 - V features split hi/lo per 4-group chunk so PE transposes start
   ~0.5us earlier; U-side split runs on ACT/Pool in parallel.
 - VT stays group-major: col (j, p) <-> target m = 16 p + j.  The
   matmul moving AP [44, p-block, j] emits columns in m order; the
   stride costs ~2x matmul throughput but every alternative (strided
   copies, DMA scatter) measured worse.

Numerics (vs jax reference):
 - l1 = (k^2-k-4b^2)/D, l2 = ((k-1)k^2 - ks - s/2)/D^2, D = k^2-4b^2,
   s = 4b^2 (exact ratios; exp(-EPS) ~ 1-1e-6 dropped).
 - |gamma1|^2 == 1 exactly => kappa_a.Ex_a = k*l1.
 - l2 * sum(dVdiag) == 0 (unit gammas) => l2 dropped from UF[5:8].
 - LN_2PI cancels between c_b and -c_a => dropped from both.
 - G3 := -gamma3 (only quadratic uses; sign-insensitive).
 - Sin HW domain is [-pi,pi]: cos(x) = sin(pi/2 - |x|).
"""

import sys

import numpy as np

sys.path.insert(0, "/opt/trn_rl_repo")

import concourse.bass as bass  # noqa: E402,F401
import concourse.mybir as mybir  # noqa: E402
import concourse.tile as tile  # noqa: E402
from concourse import bacc  # noqa: E402
from concourse.masks import make_identity  # noqa: E402

F32 = mybir.dt.float32
BF16 = mybir.dt.bfloat16
AF = mybir.ActivationFunctionType
ALU = mybir.AluOpType

N = 2048
M = 2048
NCORES = 8
NS = N // NCORES  # 256 pred rows per core
K = 11  # fp32 feature rank
K4 = 4 * K  # bf16 hi/lo doubled contraction
GP = NS // 128  # pred row-groups (2)
GT = M // 128  # target row-groups (16)
G = GP + GT  # 18

PI = float(np.pi)
EPS = 1e-6


def _body(tc, pred, targ, out):
    nc = tc.nc
    with (
        tc.tile_pool(name="main", bufs=1) as pool,
        tc.tile_pool(name="vt_psum", bufs=4, space="PSUM") as vpp,
        tc.tile_pool(name="ut_psum", bufs=1, space="PSUM") as upp,
        tc.tile_pool(name="out_psum", bufs=3, space="PSUM") as opp,
    ):
        def t(shape, tag, dtype=F32):
            return pool.tile([128, *shape], dtype, name=tag, tag=tag)

        dve = nc.vector
        act = nc.scalar
        gps = nc.gpsimd

        # ---- input DMAs first: pred partition p holds rows 2p,2p+1; targ
        # partition p holds rows 16p..16p+15 (per-partition contiguous).
        params = t([G * 5], "params")
        nc.sync.dma_start(
            out=params[:, 0 : GP * 5],
            in_=pred.rearrange("(p j) c -> p (j c)", p=128),
        )
        act.dma_start(
            out=params[:, GP * 5 : G * 5],
            in_=targ.rearrange("(p j) c -> p (j c)", p=128),
        )

        P5 = params.rearrange("p (g c) -> p c g", c=5)  # [128, 5, 18]
        kap = P5[:, 3, :]
        bet = P5[:, 4, :]
        kap_p, bet_p = kap[:, 0:GP], bet[:, 0:GP]
        kap_t, bet_t = kap[:, GP:G], bet[:, GP:G]

        # ---- constants (overlap the DMA transfer)
        half_pi = pool.tile([128, 1], F32, name="half_pi", tag="half_pi")
        gps.memset(half_pi, PI / 2)
        eps_c = pool.tile([128, 1], F32, name="eps_c", tag="eps_c")
        gps.memset(eps_c, EPS)
        one_c = pool.tile([128, 1], F32, name="one_c", tag="one_c")
        gps.memset(one_c, 1.0)
        ident = pool.tile([128, 128], BF16, name="ident", tag="ident")
        make_identity(nc, ident)

        # dummy sin: the trig ACT table loads during the input DMA
        dmy = pool.tile([128, 1], F32, name="dmy", tag="dmy")
        act.activation(dmy[:], half_pi[:], AF.Sin)

        # ---- workspace W: slot axis x 18 group columns (see v3 map)
        S = 62
        W = t([S, G], "W")
        gps.memset(W[:, 47, :], 1.0)  # VF0 = 1

        # ---- Pool: kappa/beta + rec-independent U-chain (needs only DMA)
        U = t([20, GP], "U")
        b2p = W[:, 61, 0:GP]
        one_b = one_c.broadcast_to([128, GP])

        def u(i):
            return U[:, i, :]

        gps.tensor_add(W[:, 61, :], bet, bet)  # b2 = 2*beta
        gps.tensor_sub(W[:, 43, :], kap, W[:, 61, :])  # km
        gps.tensor_add(W[:, 44, :], kap, W[:, 61, :])  # kp
        gps.tensor_mul(W[:, 45, :], W[:, 43, :], W[:, 44, :])  # LNIN
        gps.tensor_mul(u(0), kap_p, kap_p)  # x2 = k^2
        gps.tensor_mul(u(1), b2p, b2p)  # s = 4b^2
        gps.tensor_sub(u(2), u(0), u(1))  # D = k^2 - s
        gps.tensor_sub(u(5), u(2), kap_p)  # n2 = k^2-k-s
        gps.tensor_sub(u(7), kap_p, one_b)  # k-1
        gps.tensor_mul(u(8), u(7), u(0))  # t2 = (k-1)k^2
        gps.tensor_mul(u(9), kap_p, u(1))  # ks
        gps.tensor_mul(u(10), bet_p, b2p)  # s/2 = 2b^2
        gps.tensor_sub(u(11), u(8), u(9))
        gps.tensor_sub(u(12), u(11), u(10))  # Qh = t2 - ks - s/2

        # ---- trig (ACT) ----
        angles = P5[:, 0:3, :]
        absv = W[:, 58:61, :]
        dve.scalar_tensor_tensor(absv, angles, -1.0, angles, ALU.mult, ALU.max)
        act.activation(W[:, 3:6, :], angles, AF.Sin)  # se, sa, sp
        act.activation(W[:, 0:3, :], absv, AF.Sin, bias=half_pi, scale=-1.0)
        # ln table loads here (once); lnprod = ln((k-2b)(k+2b) + EPS)
        act.activation(W[:, 46, :], W[:, 45, :], AF.Ln, bias=eps_c)

        def rep_outer(ap, n):  # [128, a, g] -> [128, a, n, g]
            return ap.unsqueeze(2).broadcast_to([128, ap.shape[1], n, ap.shape[2]])

        def rep_inner(ap, n):  # [128, b, g] -> [128, n, b, g]
            return ap.unsqueeze(1).broadcast_to([128, n, ap.shape[1], ap.shape[2]])

        # ---- DVE gamma chain ----
        cpsp = W[:, 2:6:3, :]  # [cp, sp]
        cese = W[:, 0:4:3, :]  # [ce, se]
        sa_b = W[:, 4:5, :].broadcast_to([128, 2, G])
        dve.tensor_mul(W[:, 9:11, :], cpsp, W[:, 1:2, :].broadcast_to([128, 2, G]))
        dve.tensor_copy(W[:, 6:7, :], W[:, 1:2, :])  # g1x = ca
        dve.tensor_mul(W[:, 7:9, :], sa_b, cese)  # g1y, g1z
        aux1_out = W[:, 11:15, :].rearrange("p (a b) g -> p a b g", a=2)
        dve.tensor_mul(aux1_out, rep_outer(W[:, 5:1:-3, :], 2), rep_inner(cese, 2))
        aux2_out = W[:, 15:19, :].rearrange("p (a b) g -> p a b g", a=2)
        dve.tensor_mul(aux2_out, rep_outer(W[:, 9:11, :], 2), rep_inner(cese, 2))
        dve.scalar_tensor_tensor(W[:, 19:23:3, :], cpsp, -1.0, sa_b, ALU.mult, ALU.mult)
        dve.tensor_sub(W[:, 20:25:4, :], W[:, 15:19:3, :], W[:, 12:14, :])
        dve.tensor_add(W[:, 21:24:2, :], W[:, 16:18, :], W[:, 11:15:3, :])
        # U reciprocal slotted here: its Pool-side input D is long ready
        dve.reciprocal(u(3), u(2))

        # ---- V pair products (target columns) ----
        g6t = W[:, 19:25, GP:G]
        dve.tensor_mul(W[:, 25:31, GP:G], g6t, g6t)
        offa_in1 = W[:, 19:25, GP:G].rearrange("p (a b) g -> p a b g", a=2)[:, :, 1:3, :]
        offa_out = W[:, 31:37, GP:G].rearrange("p (a b) g -> p a b g", a=2)[:, :, 0:2, :]
        dve.tensor_mul(offa_out, rep_outer(W[:, 19:25:3, GP:G], 2), offa_in1)
        dve.tensor_mul(W[:, 33:39:3, GP:G], W[:, 20:26:3, GP:G], W[:, 21:27:3, GP:G])
        dve.tensor_sub(W[:, 37:40, GP:G], W[:, 28:31, GP:G], W[:, 25:28, GP:G])
        dve.tensor_sub(W[:, 40:43, GP:G], W[:, 34:37, GP:G], W[:, 31:34, GP:G])

        # ---- Pool: rec-dependent U-chain + p1 products + U features ----
        gps.tensor_mul(u(6), u(5), u(3))  # l1
        gps.tensor_mul(u(13), u(3), u(3))  # rec^2
        gps.tensor_mul(u(14), u(12), u(13))  # l2
        gps.tensor_sub(u(15), u(6), u(14))  # dE
        gps.tensor_mul(u(16), kap_p, u(6))  # kadot = k*l1
        g1p = W[:, 6:9, 0:GP]
        gps.tensor_mul(W[:, 25:28, 0:GP], g1p, g1p)
        gps.tensor_mul(
            W[:, 28:30, 0:GP],
            W[:, 6:7, 0:GP].broadcast_to([128, 2, GP]),
            W[:, 7:9, 0:GP],
        )
        gps.tensor_mul(W[:, 30, 0:GP], W[:, 7, 0:GP], W[:, 8, 0:GP])

        UF = t([K, GP], "UF")
        gps.memset(UF[:, 1, :], 1.0)
        l1b = U[:, 6:7, :].broadcast_to([128, 3, GP])
        deb = U[:, 15:16, :].broadcast_to([128, 3, GP])
        de2 = U[:, 15:16, :].broadcast_to([128, 2, GP])
        gps.tensor_mul(UF[:, 2:5, :], g1p, l1b)
        gps.tensor_mul(UF[:, 5:8, :], W[:, 25:28, 0:GP], deb)
        gps.tensor_mul(UF[:, 8:10, :], W[:, 28:30, 0:GP], de2)
        gps.tensor_mul(UF[:, 10, :], W[:, 30, 0:GP], U[:, 15, :])
        # A = 0.5*lnprod - k + k*l1 (a1 on DVE; LN_2PI dropped)
        dve.scalar_tensor_tensor(
            u(18), W[:, 46, 0:GP], 0.5, kap_p, ALU.mult, ALU.subtract
        )
        gps.tensor_add(UF[:, 0, :], u(18), u(16))

        # ---- U hi/lo split: U44 = [Uh; Ul; Uh; Ul] ----
        VH = t([K4, GT], "VH", BF16)
        UH = t([K4, GP], "UH", BF16)
        uh2 = UH.rearrange("p (r k) g -> p r k g", r=2)
        act.copy(uh2[:, :, 0:K, :], rep_inner(UF[:], 2))
        gps.tensor_sub(
            uh2[:, :, K : 2 * K, :], rep_inner(UF[:], 2), rep_inner(UH[:, 0:K, :], 2)
        )

        # ---- V features (fp32) into W[47:58] ----
        kt3 = kap_t.unsqueeze(1).broadcast_to([128, 3, GT])
        bt3 = bet_t.unsqueeze(1).broadcast_to([128, 3, GT])
        dve.scalar_tensor_tensor(
            W[:, 48, GP:G], W[:, 46, GP:G], -0.5, kap_t, ALU.mult, ALU.add
        )
        dve.scalar_tensor_tensor(
            W[:, 49:52, GP:G], W[:, 6:9, GP:G], -1.0, kt3, ALU.mult, ALU.mult
        )
        dve.tensor_mul(W[:, 52:55, GP:G], W[:, 37:40, GP:G], bt3)
        dve.scalar_tensor_tensor(
            W[:, 55:58, GP:G], W[:, 40:43, GP:G], 2.0, bt3, ALU.mult, ALU.mult
        )

        # ---- PE: U transposes + UT interleave copy (UT col = pred row) ----
        utp = upp.tile([K4, 1024], BF16, name="utp", tag="utp")
        UT = pool.tile([K4, NS], BF16, name="UT", tag="UT")
        for j in range(GP):
            nc.tensor.transpose(utp[:, j * 128 : (j + 1) * 128], UH[:, :, j], ident[:])
        act.copy(
            UT.rearrange("k (p j) -> k j p", j=GP),
            utp[:, 0 : GP * 128].rearrange("k (j p) -> k j p", p=128),
        )

        # ---- V hi/lo split per 4-group chunk: V44 = [Vh; Vh; Vl; Vl] ----
        for q in range(4):
            gs = GP + 4 * q
            VFq = W[:, 47:58, gs : gs + 4]
            vh_hi = VH[:, 0 : 2 * K, 4 * q : 4 * q + 4].rearrange(
                "p (r k) g -> p r k g", r=2
            )
            vh_lo = VH[:, 2 * K : 4 * K, 4 * q : 4 * q + 4].rearrange(
                "p (r k) g -> p r k g", r=2
            )
            act.copy(vh_hi, rep_inner(VFq, 2))
            dve.tensor_sub(
                vh_lo, rep_inner(VFq, 2), rep_inner(VH[:, 0:K, 4 * q : 4 * q + 4], 2)
            )

        # ---- V transposes (PE) + contiguous psum->SBUF copies ----
        VT = pool.tile([K4, M], BF16, name="VT", tag="VT")
        for q in range(4):
            vtp = vpp.tile([K4, 1024], BF16, name="vtp", tag="vtp")
            for jj in range(4):
                j = q * 4 + jj
                nc.tensor.transpose(
                    vtp[:, jj * 128 : (jj + 1) * 128], VH[:, :, j], ident[:]
                )
            if q % 2 == 0:
                dve.tensor_copy(VT[:, q * 512 : (q + 1) * 512], vtp[:, 0:512])
            else:
                act.copy(VT[:, q * 512 : (q + 1) * 512], vtp[:, 0:512])

        # ---- main matmuls (bf16, K=44): moving AP emits cols in m order ----
        VTv = VT.rearrange("k (j p) -> k p j", p=128)  # col m = 16p + j
        outv = out.rearrange("(t p) m -> p t m", p=128)  # row = 128 t + p
        ci = 0
        for c in range(4):
            for ti in range(GP):
                ops = opp.tile([128, 512], F32, name="ops", tag="ops")
                nc.tensor.matmul(
                    ops[:],
                    UT[:, 128 * ti : 128 * (ti + 1)],
                    VTv[:, 32 * c : 32 * (c + 1), :],
                    start=True,
                    stop=True,
                )
                out_sb = pool.tile(
                    [128, 512], F32, name="out_sb", tag="out_sb", bufs=4
                )
                if ci % 2 == 0:
                    dve.tensor_copy(out_sb[:], ops[:])
                else:
                    act.copy(out_sb[:], ops[:])
                ring = nc.sync if ci % 2 == 0 else act
                ring.dma_start(
                    out=outv[:, ti, 512 * c : 512 * (c + 1)], in_=out_sb[:]
                )
                ci += 1


def build():
    nc = bacc.Bacc()
    pred = nc.dram_tensor("pred", [NS, 5], F32, kind="ExternalInput")
    targ = nc.dram_tensor("targ", [M, 5], F32, kind="ExternalInput")
    out = nc.dram_tensor("out", [NS, M], F32, kind="ExternalOutput")
    with tile.TileContext(nc) as tc:
        _body(tc, pred[:], targ[:], out[:])
    nc.finalize()
    return nc


_NC_CACHE = None


def _get_nc():
    global _NC_CACHE
    if _NC_CACHE is None:
        _NC_CACHE = build()
    return _NC_CACHE


def kernel(kent_pred, kent_target, trace=False, tmpdir=None):
    from concourse.bass_utils import run_bass_kernel_spmd

    nc = _get_nc()
    kent_pred = np.ascontiguousarray(np.asarray(kent_pred, dtype=np.float32))
    kent_target = np.ascontiguousarray(np.asarray(kent_target, dtype=np.float32))
    in_maps = [
        {"pred": kent_pred[i * NS : (i + 1) * NS], "targ": kent_target}
        for i in range(NCORES)
    ]
    res = run_bass_kernel_spmd(
        nc, in_maps, core_ids=list(range(NCORES)), trace=trace, tmpdir=tmpdir
    )
    out = np.concatenate([r["out"] for r in res.results], axis=0)
    if trace:
        kernel.last_results = res
    return out


# revision 8
# speedup vs baseline: 1.0994x; 1.0845x over previous
"""Kent-distribution pairwise KLD loss kernel for Trainium2 (8 NeuronCores).

v5: bf16 single-pass matmul with exact hi/lo (split-float) features,
PE pstate warm-up, and balanced DVE/Pool/ACT scheduling.

KLD[N, M] factors exactly as rank-11 U @ V^T.  One bf16 matmul with
K=44 replaces the fp32 matmul: each fp32 feature x is bf16 hi + lo
(hi+lo == x to ~2^-17) and

  U44 = [Uh; Ul; Uh; Ul],  V44 = [Vh; Vh; Vl; Vl]
  sum_k U44[k] V44[k] = sum_f (Uh+Ul)(Vh+Vl) = U . V

Trace-driven scheduling:
 - The PE idles ~10us before the first transpose and then runs at its
   LOW pstate (1.54 ns/col) for the whole kernel.  A stream of dummy
   matmuls (K=1, garbage data, recycled out-psum banks) keeps the PE
   continuously busy from kernel start so the real transposes/matmuls
   run at the ramped clock.
 - One ACT table set resident at a time: ACT runs sin sin cos [ln
   load] ln, then only table-free COPYs.
 - U-side scalar chain runs on DVE via fused scalar_tensor_tensor
   (Pool lacks TensorScalarPtr and costs ~200ns/op vs DVE ~135).
 - VT copies are 16 fine [44, 4x32] chunks so matmul c waits only on
   its own column slices.
 - VT stays group-major (col (j, p) <-> m = 16p + j); the matmul
   moving AP emits columns in m order.
 - U transposes write PSUM with a stride-2 column AP so UT (col =
   pred row 2p+j) is a contiguous copy.

Numerics identical to v2 (rel err ~9e-6 vs the jax reference).
"""

import sys

import numpy as np

sys.path.insert(0, "/opt/trn_rl_repo")

import concourse.bass as bass  # noqa: E402,F401
import concourse.mybir as mybir  # noqa: E402
import concourse.tile as tile  # noqa: E402
from concourse import bacc  # noqa: E402
from concourse.masks import make_identity  # noqa: E402

F32 = mybir.dt.float32
BF16 = mybir.dt.bfloat16
AF = mybir.ActivationFunctionType
ALU = mybir.AluOpType

N = 2048
M = 2048
NCORES = 8
NS = N // NCORES
K = 11
K4 = 4 * K
GP = NS // 128  # 2
GT = M // 128  # 16
G = GP + GT  # 18

PI = float(np.pi)
EPS = 1e-6
N_WARMUP = 21  # dummy matmuls to hold the PE at its ramped pstate


def _body(tc, pred, targ, out):
    nc = tc.nc
    with (
        tc.tile_pool(name="main", bufs=1) as pool,
        tc.tile_pool(name="vt_psum", bufs=4, space="PSUM") as vpp,
        tc.tile_pool(name="ut_psum", bufs=1, space="PSUM") as upp,
        tc.tile_pool(name="out_psum", bufs=3, space="PSUM") as opp,
    ):
        def t(shape, tag, dtype=F32):
            return pool.tile([128, *shape], dtype, name=tag, tag=tag)

        dve = nc.vector
        act = nc.scalar
        gps = nc.gpsimd

        # ---- input DMAs first ----
        params = t([G * 5], "params")
        nc.sync.dma_start(
            out=params[:, 0 : GP * 5],
            in_=pred.rearrange("(p j) c -> p (j c)", p=128),
        )
        act.dma_start(
            out=params[:, GP * 5 : G * 5],
            in_=targ.rearrange("(p j) c -> p (j c)", p=128),
        )

        P5 = params.rearrange("p (g c) -> p c g", c=5)
        kap = P5[:, 3, :]
        bet = P5[:, 4, :]
        kap_p, bet_p = kap[:, 0:GP], bet[:, 0:GP]
        kap_t, bet_t = kap[:, GP:G], bet[:, GP:G]

        # ---- constants ----
        half_pi = pool.tile([128, 1], F32, name="half_pi", tag="half_pi")
        gps.memset(half_pi, PI / 2)
        eps_c = pool.tile([128, 1], F32, name="eps_c", tag="eps_c")
        gps.memset(eps_c, EPS)
        dummy_sb = pool.tile([1, 512], BF16, name="dummy_sb", tag="dummy_sb")
        gps.memset(dummy_sb, 0.0)
        ident = pool.tile([128, 128], BF16, name="ident", tag="ident")
        make_identity(nc, ident)

        # dummy sin: trig table loads during the input DMA
        dmy = pool.tile([128, 1], F32, name="dmy", tag="dmy")
        act.activation(dmy[:], half_pi[:], AF.Sin)

        # ---- PE warm-up: keep the tensor engine busy from t~6.5us so the
        # pstate governor ramps the clock before the real transposes/matmuls
        for _ in range(N_WARMUP):
            wps = opp.tile([128, 512], F32, name="wps", tag="ops")
            nc.tensor.matmul(
                wps[:], dummy_sb[0:1, 0:128], dummy_sb[0:1, 0:512],
                start=True, stop=True,
            )

        # ---- workspace W (see slot map in v3/v4) ----
        S = 62
        W = t([S, G], "W")
        gps.memset(W[:, 47, :], 1.0)  # VF0 = 1

        U = t([20, GP], "U")

        def u(i):
            return U[:, i, :]

        # ---- Pool: kappa/beta shared (needs only the DMA) ----
        gps.tensor_add(W[:, 61, :], bet, bet)  # b2 = 2*beta
        gps.tensor_sub(W[:, 43, :], kap, W[:, 61, :])  # km
        gps.tensor_add(W[:, 44, :], kap, W[:, 61, :])  # kp
        gps.tensor_mul(W[:, 45, :], W[:, 43, :], W[:, 44, :])  # LNIN

        # ---- DVE: abs + U-chain head (only needs the DMA) ----
        angles = P5[:, 0:3, :]
        absv = W[:, 58:61, :]
        dve.scalar_tensor_tensor(absv, angles, -1.0, angles, ALU.mult, ALU.max)
        dve.tensor_mul(u(0), kap_p, kap_p)  # x2
        dve.scalar_tensor_tensor(u(1), bet_p, 4.0, bet_p, ALU.mult, ALU.mult)  # s
        dve.tensor_sub(u(2), u(0), u(1))  # D
        dve.reciprocal(u(3), u(2))  # rec

        # ---- ACT: trig + ln (one ln-table load in between) ----
        act.activation(W[:, 3:6, :], angles, AF.Sin)  # se, sa, sp
        act.activation(W[:, 0:3, :], absv, AF.Sin, bias=half_pi, scale=-1.0)
        act.activation(W[:, 46, :], W[:, 45, :], AF.Ln, bias=eps_c)  # lnprod

        def rep_outer(ap, n):
            return ap.unsqueeze(2).broadcast_to([128, ap.shape[1], n, ap.shape[2]])

        def rep_inner(ap, n):
            return ap.unsqueeze(1).broadcast_to([128, n, ap.shape[1], ap.shape[2]])

        # ---- DVE: gamma chain ----
        cpsp = W[:, 2:6:3, :]
        cese = W[:, 0:4:3, :]
        sa_b = W[:, 4:5, :].broadcast_to([128, 2, G])
        dve.tensor_mul(W[:, 9:11, :], cpsp, W[:, 1:2, :].broadcast_to([128, 2, G]))
        dve.tensor_copy(W[:, 6:7, :], W[:, 1:2, :])
        dve.tensor_mul(W[:, 7:9, :], sa_b, cese)
        aux1_out = W[:, 11:15, :].rearrange("p (a b) g -> p a b g", a=2)
        dve.tensor_mul(aux1_out, rep_outer(W[:, 5:1:-3, :], 2), rep_inner(cese, 2))
        aux2_out = W[:, 15:19, :].rearrange("p (a b) g -> p a b g", a=2)
        dve.tensor_mul(aux2_out, rep_outer(W[:, 9:11, :], 2), rep_inner(cese, 2))
        dve.scalar_tensor_tensor(W[:, 19:23:3, :], cpsp, -1.0, sa_b, ALU.mult, ALU.mult)
        dve.tensor_sub(W[:, 20:25:4, :], W[:, 15:19:3, :], W[:, 12:14, :])
        dve.tensor_add(W[:, 21:24:2, :], W[:, 16:18, :], W[:, 11:15:3, :])

        # ---- Pool: p1 products then V pair products ----
        g1p = W[:, 6:9, 0:GP]
        gps.tensor_mul(W[:, 25:28, 0:GP], g1p, g1p)
        gps.tensor_mul(
            W[:, 28:30, 0:GP],
            W[:, 6:7, 0:GP].broadcast_to([128, 2, GP]),
            W[:, 7:9, 0:GP],
        )
        gps.tensor_mul(W[:, 30, 0:GP], W[:, 7, 0:GP], W[:, 8, 0:GP])
        g6t = W[:, 19:25, GP:G]
        gps.tensor_mul(W[:, 25:31, GP:G], g6t, g6t)
        offa_in1 = W[:, 19:25, GP:G].rearrange("p (a b) g -> p a b g", a=2)[:, :, 1:3, :]
        offa_out = W[:, 31:37, GP:G].rearrange("p (a b) g -> p a b g", a=2)[:, :, 0:2, :]
        gps.tensor_mul(offa_out, rep_outer(W[:, 19:25:3, GP:G], 2), offa_in1)
        gps.tensor_mul(W[:, 33:39:3, GP:G], W[:, 20:26:3, GP:G], W[:, 21:27:3, GP:G])

        # ---- DVE: U-chain middle + dV subs ----
        dve.tensor_sub(u(5), u(2), kap_p)  # n2 = D - k
        dve.tensor_mul(u(6), u(5), u(3))  # l1
        dve.scalar_tensor_tensor(u(7), kap_p, -1.0, u(0), ALU.add, ALU.mult)  # t2
        dve.scalar_tensor_tensor(u(8), kap_p, 0.5, u(1), ALU.add, ALU.mult)  # ks+s/2
        dve.tensor_sub(u(9), u(7), u(8))  # Qh
        dve.tensor_sub(W[:, 37:40, GP:G], W[:, 28:31, GP:G], W[:, 25:28, GP:G])
        dve.tensor_sub(W[:, 40:43, GP:G], W[:, 34:37, GP:G], W[:, 31:34, GP:G])
        dve.tensor_mul(u(10), u(3), u(3))  # rec^2
        dve.tensor_mul(u(11), u(9), u(10))  # l2
        dve.tensor_sub(u(12), u(6), u(11))  # dE
        dve.tensor_mul(u(13), kap_p, u(6))  # kadot

        # ---- DVE: V features (stt) ; Pool: VF[5:8] ----
        kt3 = kap_t.unsqueeze(1).broadcast_to([128, 3, GT])
        bt3 = bet_t.unsqueeze(1).broadcast_to([128, 3, GT])
        dve.scalar_tensor_tensor(
            W[:, 48, GP:G], W[:, 46, GP:G], -0.5, kap_t, ALU.mult, ALU.add
        )
        dve.scalar_tensor_tensor(
            W[:, 49:52, GP:G], W[:, 6:9, GP:G], -1.0, kt3, ALU.mult, ALU.mult
        )
        gps.tensor_mul(W[:, 52:55, GP:G], W[:, 37:40, GP:G], bt3)
        dve.scalar_tensor_tensor(
            W[:, 55:58, GP:G], W[:, 40:43, GP:G], 2.0, bt3, ALU.mult, ALU.mult
        )
        # A = 0.5*lnprod - k + k*l1
        dve.scalar_tensor_tensor(
            u(14), W[:, 46, 0:GP], 0.5, kap_p, ALU.mult, ALU.subtract
        )

        # ---- Pool: U features + split; UF tile ----
        UF = t([K, GP], "UF")
        gps.memset(UF[:, 1, :], 1.0)
        l1b = U[:, 6:7, :].broadcast_to([128, 3, GP])
        deb = U[:, 12:13, :].broadcast_to([128, 3, GP])
        de2 = U[:, 12:13, :].broadcast_to([128, 2, GP])
        gps.tensor_mul(UF[:, 2:5, :], g1p, l1b)
        gps.tensor_mul(UF[:, 5:8, :], W[:, 25:28, 0:GP], deb)
        gps.tensor_mul(UF[:, 8:10, :], W[:, 28:30, 0:GP], de2)
        gps.tensor_mul(UF[:, 10, :], W[:, 30, 0:GP], U[:, 12, :])
        gps.tensor_add(UF[:, 0, :], u(14), u(13))

        VH = t([K4, GT], "VH", BF16)
        UH = t([K4, GP], "UH", BF16)
        uh2 = UH.rearrange("p (r k) g -> p r k g", r=2)
        gps.tensor_copy(uh2[:, :, 0:K, :], rep_inner(UF[:], 2))
        gps.tensor_sub(
            uh2[:, :, K : 2 * K, :], rep_inner(UF[:], 2), rep_inner(UH[:, 0:K, :], 2)
        )

        # ---- V hi/lo split per 4-group chunk (hi on ACT, lo on DVE) ----
        for q in range(4):
            gs = GP + 4 * q
            VFq = W[:, 47:58, gs : gs + 4]
            vh_hi = VH[:, 0 : 2 * K, 4 * q : 4 * q + 4].rearrange(
                "p (r k) g -> p r k g", r=2
            )
            vh_lo = VH[:, 2 * K : 4 * K, 4 * q : 4 * q + 4].rearrange(
                "p (r k) g -> p r k g", r=2
            )
            act.copy(vh_hi, rep_inner(VFq, 2))
            dve.tensor_sub(
                vh_lo, rep_inner(VFq, 2), rep_inner(VH[:, 0:K, 4 * q : 4 * q + 4], 2)
            )

        # ---- PE: V transposes; fine-grained [44, 4x32] copies per (q, c) ----
        VT = pool.tile([K4, M], BF16, name="VT", tag="VT")
        ci = 0
        for q in range(4):
            vtp = vpp.tile([K4, 1024], BF16, name="vtp", tag="vtp")
            for jj in range(4):
                j = q * 4 + jj
                nc.tensor.transpose(
                    vtp[:, jj * 128 : (jj + 1) * 128], VH[:, :, j], ident[:]
                )
            # copy (q, c): cols [32c, 32c+32) of each of the 4 groups
            src4 = vtp[:, 0:512].rearrange("k (j p) -> k j p", p=128)
            dst4 = VT[:, 512 * q : 512 * (q + 1)].rearrange(
                "k (j p) -> k j p", p=128
            )
            for c in range(4):
                if ci % 2 == 0:
                    dve.tensor_copy(
                        dst4[:, :, 32 * c : 32 * (c + 1)],
                        src4[:, :, 32 * c : 32 * (c + 1)],
                    )
                else:
                    act.copy(
                        dst4[:, :, 32 * c : 32 * (c + 1)],
                        src4[:, :, 32 * c : 32 * (c + 1)],
                    )
                ci += 1

        # ---- PE: U transposes + interleave copy (UT col = pred row 2p+j) ----
        utp = upp.tile([K4, 1024], BF16, name="utp", tag="utp")
        UT = pool.tile([K4, NS], BF16, name="UT", tag="UT")
        for j in range(GP):
            nc.tensor.transpose(utp[:, j * 128 : (j + 1) * 128], UH[:, :, j], ident[:])
        act.copy(
            UT.rearrange("k (p j) -> k j p", j=GP),
            utp[:, 0 : GP * 128].rearrange("k (j p) -> k j p", p=128),
        )

        # ---- main matmuls (bf16, K=44) ----
        VTv = VT.rearrange("k (j p) -> k p j", p=128)  # col m = 16p + j
        outv = out.rearrange("(t p) m -> p t m", p=128)
        ci = 0
        for c in range(4):
            for ti in range(GP):
                ops = opp.tile([128, 512], F32, name="ops", tag="ops")
                nc.tensor.matmul(
                    ops[:],
                    UT[:, 128 * ti : 128 * (ti + 1)],
                    VTv[:, 32 * c : 32 * (c + 1), :],
                    start=True,
                    stop=True,
                )
                out_sb = pool.tile(
                    [128, 512], F32, name="out_sb", tag="out_sb", bufs=4
                )
                if ci % 2 == 0:
                    dve.tensor_copy(out_sb[:], ops[:])
                else:
                    act.copy(out_sb[:], ops[:])
                ring = nc.sync if ci % 2 == 0 else act
                ring.dma_start(
                    out=outv[:, ti, 512 * c : 512 * (c + 1)], in_=out_sb[:]
                )
                ci += 1


def build():
    nc = bacc.Bacc()
    pred = nc.dram_tensor("pred", [NS, 5], F32, kind="ExternalInput")
    targ = nc.dram_tensor("targ", [M, 5], F32, kind="ExternalInput")
    out = nc.dram_tensor("out", [NS, M], F32, kind="ExternalOutput")
    with tile.TileContext(nc) as tc:
        _body(tc, pred[:], targ[:], out[:])
    nc.finalize()
    return nc


_NC_CACHE = None


def _get_nc():
    global _NC_CACHE
    if _NC_CACHE is None:
        _NC_CACHE = build()
    return _NC_CACHE


def kernel(kent_pred, kent_target, trace=False, tmpdir=None):
    from concourse.bass_utils import run_bass_kernel_spmd

    nc = _get_nc()
    kent_pred = np.ascontiguousarray(np.asarray(kent_pred, dtype=np.float32))
    kent_target = np.ascontiguousarray(np.asarray(kent_target, dtype=np.float32))
    in_maps = [
        {"pred": kent_pred[i * NS : (i + 1) * NS], "targ": kent_target}
        for i in range(NCORES)
    ]
    res = run_bass_kernel_spmd(
        nc, in_maps, core_ids=list(range(NCORES)), trace=trace, tmpdir=tmpdir
    )
    out = np.concatenate([r["out"] for r in res.results], axis=0)
    if trace:
        kernel.last_results = res
    return out


# revision 9
# speedup vs baseline: 1.2212x; 1.1108x over previous
"""Kent-distribution pairwise KLD loss kernel for Trainium2 (8 NeuronCores).

v5: bf16 single-pass matmul with exact hi/lo (split-float) features,
PE pstate warm-up, and balanced DVE/Pool/ACT scheduling.

KLD[N, M] factors exactly as rank-11 U @ V^T.  One bf16 matmul with
K=44 replaces the fp32 matmul: each fp32 feature x is bf16 hi + lo
(hi+lo == x to ~2^-17) and

  U44 = [Uh; Ul; Uh; Ul],  V44 = [Vh; Vh; Vl; Vl]
  sum_k U44[k] V44[k] = sum_f (Uh+Ul)(Vh+Vl) = U . V

Trace-driven scheduling:
 - The PE idles ~10us before the first transpose and then runs at its
   LOW pstate (1.54 ns/col) for the whole kernel.  A stream of dummy
   matmuls (K=1, garbage data, recycled out-psum banks) keeps the PE
   continuously busy from kernel start so the real transposes/matmuls
   run at the ramped clock.
 - One ACT table set resident at a time: ACT runs sin sin cos [ln
   load] ln, then only table-free COPYs.
 - U-side scalar chain runs on DVE via fused scalar_tensor_tensor
   (Pool lacks TensorScalarPtr and costs ~200ns/op vs DVE ~135).
 - VT copies are 16 fine [44, 4x32] chunks so matmul c waits only on
   its own column slices.
 - VT stays group-major (col (j, p) <-> m = 16p + j); the matmul
   moving AP emits columns in m order.
 - U transposes write PSUM with a stride-2 column AP so UT (col =
   pred row 2p+j) is a contiguous copy.

Numerics identical to v2 (rel err ~9e-6 vs the jax reference).
"""

import sys

import numpy as np

sys.path.insert(0, "/opt/trn_rl_repo")

import concourse.bass as bass  # noqa: E402,F401
import concourse.mybir as mybir  # noqa: E402
import concourse.tile as tile  # noqa: E402
from concourse import bacc  # noqa: E402
from concourse.masks import make_identity  # noqa: E402

F32 = mybir.dt.float32
BF16 = mybir.dt.bfloat16
AF = mybir.ActivationFunctionType
ALU = mybir.AluOpType

N = 2048
M = 2048
NCORES = 8
NS = N // NCORES
K = 11
K4 = 4 * K
GP = NS // 128  # 2
GT = M // 128  # 16
G = GP + GT  # 18

PI = float(np.pi)
EPS = 1e-6
N_WARMUP = 12  # dummy matmuls to keep the PE warm until ~13.2us (input
# arrives ~10, VH ~14.4); sized to never delay the real transposes


def _body(tc, pred, targ, out):
    nc = tc.nc
    with (
        tc.tile_pool(name="main", bufs=1) as pool,
        tc.tile_pool(name="vt_psum", bufs=4, space="PSUM") as vpp,
        tc.tile_pool(name="ut_psum", bufs=1, space="PSUM") as upp,
        tc.tile_pool(name="out_psum", bufs=3, space="PSUM") as opp,
    ):
        def t(shape, tag, dtype=F32):
            return pool.tile([128, *shape], dtype, name=tag, tag=tag)

        dve = nc.vector
        act = nc.scalar
        gps = nc.gpsimd

        # ---- input DMAs first ----
        params = t([G * 5], "params")
        nc.sync.dma_start(
            out=params[:, 0 : GP * 5],
            in_=pred.rearrange("(p j) c -> p (j c)", p=128),
        )
        act.dma_start(
            out=params[:, GP * 5 : G * 5],
            in_=targ.rearrange("(p j) c -> p (j c)", p=128),
        )

        P5 = params.rearrange("p (g c) -> p c g", c=5)
        kap = P5[:, 3, :]
        bet = P5[:, 4, :]
        kap_p, bet_p = kap[:, 0:GP], bet[:, 0:GP]
        kap_t, bet_t = kap[:, GP:G], bet[:, GP:G]

        # ---- constants ----
        half_pi = pool.tile([128, 1], F32, name="half_pi", tag="half_pi")
        gps.memset(half_pi, PI / 2)
        eps_c = pool.tile([128, 1], F32, name="eps_c", tag="eps_c")
        gps.memset(eps_c, EPS)
        dummy_sb = pool.tile([1, 512], BF16, name="dummy_sb", tag="dummy_sb")
        gps.memset(dummy_sb, 0.0)
        ident = pool.tile([128, 128], BF16, name="ident", tag="ident")
        make_identity(nc, ident)

        # dummy sin: trig table loads during the input DMA
        dmy = pool.tile([128, 1], F32, name="dmy", tag="dmy")
        act.activation(dmy[:], half_pi[:], AF.Sin)

        # ---- PE warm-up: keep the tensor engine busy from t~6.5us so the
        # pstate governor ramps the clock before the real transposes/matmuls
        for _ in range(N_WARMUP):
            wps = opp.tile([128, 512], F32, name="wps", tag="ops")
            nc.tensor.matmul(
                wps[:], dummy_sb[0:1, 0:128], dummy_sb[0:1, 0:512],
                start=True, stop=True,
            )

        # ---- workspace W (see slot map in v3/v4) ----
        S = 62
        W = t([S, G], "W")
        gps.memset(W[:, 47, :], 1.0)  # VF0 = 1

        U = t([20, GP], "U")

        def u(i):
            return U[:, i, :]

        # ---- Pool: kappa/beta shared (needs only the DMA) ----
        gps.tensor_add(W[:, 61, :], bet, bet)  # b2 = 2*beta
        gps.tensor_sub(W[:, 43, :], kap, W[:, 61, :])  # km
        gps.tensor_add(W[:, 44, :], kap, W[:, 61, :])  # kp
        gps.tensor_mul(W[:, 45, :], W[:, 43, :], W[:, 44, :])  # LNIN

        # ---- DVE: abs + U-chain head (only needs the DMA) ----
        angles = P5[:, 0:3, :]
        absv = W[:, 58:61, :]
        dve.scalar_tensor_tensor(absv, angles, -1.0, angles, ALU.mult, ALU.max)
        dve.tensor_mul(u(0), kap_p, kap_p)  # x2
        dve.scalar_tensor_tensor(u(1), bet_p, 4.0, bet_p, ALU.mult, ALU.mult)  # s
        dve.tensor_sub(u(2), u(0), u(1))  # D
        dve.reciprocal(u(3), u(2))  # rec

        # ---- ACT: trig + ln (one ln-table load in between) ----
        act.activation(W[:, 3:6, :], angles, AF.Sin)  # se, sa, sp
        act.activation(W[:, 0:3, :], absv, AF.Sin, bias=half_pi, scale=-1.0)
        act.activation(W[:, 46, :], W[:, 45, :], AF.Ln, bias=eps_c)  # lnprod

        def rep_outer(ap, n):
            return ap.unsqueeze(2).broadcast_to([128, ap.shape[1], n, ap.shape[2]])

        def rep_inner(ap, n):
            return ap.unsqueeze(1).broadcast_to([128, n, ap.shape[1], ap.shape[2]])

        # ---- DVE: gamma chain ----
        cpsp = W[:, 2:6:3, :]
        cese = W[:, 0:4:3, :]
        sa_b = W[:, 4:5, :].broadcast_to([128, 2, G])
        dve.tensor_mul(W[:, 9:11, :], cpsp, W[:, 1:2, :].broadcast_to([128, 2, G]))
        dve.tensor_copy(W[:, 6:7, :], W[:, 1:2, :])
        dve.tensor_mul(W[:, 7:9, :], sa_b, cese)
        aux1_out = W[:, 11:15, :].rearrange("p (a b) g -> p a b g", a=2)
        dve.tensor_mul(aux1_out, rep_outer(W[:, 5:1:-3, :], 2), rep_inner(cese, 2))
        aux2_out = W[:, 15:19, :].rearrange("p (a b) g -> p a b g", a=2)
        dve.tensor_mul(aux2_out, rep_outer(W[:, 9:11, :], 2), rep_inner(cese, 2))
        dve.scalar_tensor_tensor(W[:, 19:23:3, :], cpsp, -1.0, sa_b, ALU.mult, ALU.mult)
        dve.tensor_sub(W[:, 20:25:4, :], W[:, 15:19:3, :], W[:, 12:14, :])
        dve.tensor_add(W[:, 21:24:2, :], W[:, 16:18, :], W[:, 11:15:3, :])

        # ---- Pool: p1 products then V pair products ----
        g1p = W[:, 6:9, 0:GP]
        gps.tensor_mul(W[:, 25:28, 0:GP], g1p, g1p)
        gps.tensor_mul(
            W[:, 28:30, 0:GP],
            W[:, 6:7, 0:GP].broadcast_to([128, 2, GP]),
            W[:, 7:9, 0:GP],
        )
        gps.tensor_mul(W[:, 30, 0:GP], W[:, 7, 0:GP], W[:, 8, 0:GP])
        g6t = W[:, 19:25, GP:G]
        gps.tensor_mul(W[:, 25:31, GP:G], g6t, g6t)
        offa_in1 = W[:, 19:25, GP:G].rearrange("p (a b) g -> p a b g", a=2)[:, :, 1:3, :]
        offa_out = W[:, 31:37, GP:G].rearrange("p (a b) g -> p a b g", a=2)[:, :, 0:2, :]
        gps.tensor_mul(offa_out, rep_outer(W[:, 19:25:3, GP:G], 2), offa_in1)
        gps.tensor_mul(W[:, 33:39:3, GP:G], W[:, 20:26:3, GP:G], W[:, 21:27:3, GP:G])

        # ---- DVE: U-chain middle + dV subs ----
        dve.tensor_sub(u(5), u(2), kap_p)  # n2 = D - k
        dve.tensor_mul(u(6), u(5), u(3))  # l1
        dve.scalar_tensor_tensor(u(7), kap_p, -1.0, u(0), ALU.add, ALU.mult)  # t2
        dve.scalar_tensor_tensor(u(8), kap_p, 0.5, u(1), ALU.add, ALU.mult)  # ks+s/2
        dve.tensor_sub(u(9), u(7), u(8))  # Qh
        dve.tensor_sub(W[:, 37:40, GP:G], W[:, 28:31, GP:G], W[:, 25:28, GP:G])
        dve.tensor_sub(W[:, 40:43, GP:G], W[:, 34:37, GP:G], W[:, 31:34, GP:G])
        dve.tensor_mul(u(10), u(3), u(3))  # rec^2
        dve.tensor_mul(u(11), u(9), u(10))  # l2
        dve.tensor_sub(u(12), u(6), u(11))  # dE
        dve.tensor_mul(u(13), kap_p, u(6))  # kadot

        # ---- DVE: V features (stt) ; Pool: VF[5:8] ----
        kt3 = kap_t.unsqueeze(1).broadcast_to([128, 3, GT])
        bt3 = bet_t.unsqueeze(1).broadcast_to([128, 3, GT])
        dve.scalar_tensor_tensor(
            W[:, 48, GP:G], W[:, 46, GP:G], -0.5, kap_t, ALU.mult, ALU.add
        )
        dve.scalar_tensor_tensor(
            W[:, 49:52, GP:G], W[:, 6:9, GP:G], -1.0, kt3, ALU.mult, ALU.mult
        )
        gps.tensor_mul(W[:, 52:55, GP:G], W[:, 37:40, GP:G], bt3)
        dve.scalar_tensor_tensor(
            W[:, 55:58, GP:G], W[:, 40:43, GP:G], 2.0, bt3, ALU.mult, ALU.mult
        )
        # A = 0.5*lnprod - k + k*l1
        dve.scalar_tensor_tensor(
            u(14), W[:, 46, 0:GP], 0.5, kap_p, ALU.mult, ALU.subtract
        )

        # ---- Pool: U features + split; UF tile ----
        UF = t([K, GP], "UF")
        gps.memset(UF[:, 1, :], 1.0)
        l1b = U[:, 6:7, :].broadcast_to([128, 3, GP])
        deb = U[:, 12:13, :].broadcast_to([128, 3, GP])
        de2 = U[:, 12:13, :].broadcast_to([128, 2, GP])
        gps.tensor_mul(UF[:, 2:5, :], g1p, l1b)
        gps.tensor_mul(UF[:, 5:8, :], W[:, 25:28, 0:GP], deb)
        gps.tensor_mul(UF[:, 8:10, :], W[:, 28:30, 0:GP], de2)
        gps.tensor_mul(UF[:, 10, :], W[:, 30, 0:GP], U[:, 12, :])
        gps.tensor_add(UF[:, 0, :], u(14), u(13))

        VH = t([K4, GT], "VH", BF16)
        UH = t([K4, GP], "UH", BF16)
        uh2 = UH.rearrange("p (r k) g -> p r k g", r=2)
        gps.tensor_copy(uh2[:, :, 0:K, :], rep_inner(UF[:], 2))
        gps.tensor_sub(
            uh2[:, :, K : 2 * K, :], rep_inner(UF[:], 2), rep_inner(UH[:, 0:K, :], 2)
        )

        # ---- V hi/lo split per 4-group chunk (hi on ACT, lo on DVE) ----
        for q in range(4):
            gs = GP + 4 * q
            VFq = W[:, 47:58, gs : gs + 4]
            vh_hi = VH[:, 0 : 2 * K, 4 * q : 4 * q + 4].rearrange(
                "p (r k) g -> p r k g", r=2
            )
            vh_lo = VH[:, 2 * K : 4 * K, 4 * q : 4 * q + 4].rearrange(
                "p (r k) g -> p r k g", r=2
            )
            act.copy(vh_hi, rep_inner(VFq, 2))
            dve.tensor_sub(
                vh_lo, rep_inner(VFq, 2), rep_inner(VH[:, 0:K, 4 * q : 4 * q + 4], 2)
            )

        # ---- PE: V transposes (q0 first), U transposes, V q1-3; copies
        # chase on DVE (q0, q2) / ACT (UT, q1, q3) ----
        VT = pool.tile([K4, M], BF16, name="VT", tag="VT")
        utp = upp.tile([K4, 1024], BF16, name="utp", tag="utp")
        UT = pool.tile([K4, NS], BF16, name="UT", tag="UT")
        vtps = []
        for q in range(4):
            vtp = vpp.tile([K4, 1024], BF16, name="vtp", tag="vtp")
            vtps.append(vtp)
            for jj in range(4):
                j = q * 4 + jj
                nc.tensor.transpose(
                    vtp[:, jj * 128 : (jj + 1) * 128], VH[:, :, j], ident[:]
                )
            if q == 0:
                # U transposes early so the UT interleave copy (ACT) runs
                # before the ACT-side VT copies
                for j in range(GP):
                    nc.tensor.transpose(
                        utp[:, j * 128 : (j + 1) * 128], UH[:, :, j], ident[:]
                    )
                act.copy(
                    UT.rearrange("k (p j) -> k j p", j=GP),
                    utp[:, 0 : GP * 128].rearrange("k (j p) -> k j p", p=128),
                )
            if q % 2 == 0:
                dve.tensor_copy(VT[:, q * 512 : (q + 1) * 512], vtp[:, 0:512])
            else:
                act.copy(VT[:, q * 512 : (q + 1) * 512], vtp[:, 0:512])

        # ---- main matmuls (bf16, K=44) ----
        VTv = VT.rearrange("k (j p) -> k p j", p=128)  # col m = 16p + j
        outv = out.rearrange("(t p) m -> p t m", p=128)
        ci = 0
        for c in range(4):
            for ti in range(GP):
                ops = opp.tile([128, 512], F32, name="ops", tag="ops")
                nc.tensor.matmul(
                    ops[:],
                    UT[:, 128 * ti : 128 * (ti + 1)],
                    VTv[:, 32 * c : 32 * (c + 1), :],
                    start=True,
                    stop=True,
                )
                out_sb = pool.tile(
                    [128, 512], F32, name="out_sb", tag="out_sb", bufs=4
                )
                if ci % 2 == 0:
                    dve.tensor_copy(out_sb[:], ops[:])
                else:
                    act.copy(out_sb[:], ops[:])
                ring = nc.sync if ci % 2 == 0 else act
                ring.dma_start(
                    out=outv[:, ti, 512 * c : 512 * (c + 1)], in_=out_sb[:]
                )
                ci += 1


def build():
    nc = bacc.Bacc()
    pred = nc.dram_tensor("pred", [NS, 5], F32, kind="ExternalInput")
    targ = nc.dram_tensor("targ", [M, 5], F32, kind="ExternalInput")
    out = nc.dram_tensor("out", [NS, M], F32, kind="ExternalOutput")
    with tile.TileContext(nc) as tc:
        _body(tc, pred[:], targ[:], out[:])
    nc.finalize()
    return nc


_NC_CACHE = None


def _get_nc():
    global _NC_CACHE
    if _NC_CACHE is None:
        _NC_CACHE = build()
    return _NC_CACHE


def kernel(kent_pred, kent_target, trace=False, tmpdir=None):
    from concourse.bass_utils import run_bass_kernel_spmd

    nc = _get_nc()
    kent_pred = np.ascontiguousarray(np.asarray(kent_pred, dtype=np.float32))
    kent_target = np.ascontiguousarray(np.asarray(kent_target, dtype=np.float32))
    in_maps = [
        {"pred": kent_pred[i * NS : (i + 1) * NS], "targ": kent_target}
        for i in range(NCORES)
    ]
    res = run_bass_kernel_spmd(
        nc, in_maps, core_ids=list(range(NCORES)), trace=trace, tmpdir=tmpdir
    )
    out = np.concatenate([r["out"] for r in res.results], axis=0)
    if trace:
        kernel.last_results = res
    return out
